# revision 54
# baseline (speedup 1.0000x reference)
"""Paged-attention decode kernel for 8 TRN2 NeuronCores.

Sharding: tensor-parallel over the 8 KV heads (one per core). The host applies
the KV-cache scatter update, gathers each request's K/V context from the paged
pools (block_tables are host-visible), trims K to the exact context length and
V to full 128-position chunks (zeroing beyond ctx), and packs per-core
matmul-ready slabs:

  ktd [128=dh, TOTK]         K^T slabs, ctx-packed (request 0 padded to Wmax
                             so its start-bit QK covers every PSUM column)
  vd  [128=pos%128, slabs]   V per group of 4 requests, chunk-major
                             [chunk][request][dh] so the PV rhs is contiguous

Device kernel, built around measured TRN2 costs (each dma_start costs ~600ns
of serial issue on the sync engine; the PE streams bf16 at ~1 col/cycle; the
DMA engines sustain ~360GB/s on big static transfers; an SBUF XBAR-transpose
DMA queues behind pending input transfers, so transposes stay on the PE):

  - 16 multi-request K piece DMAs + 8 V group DMAs + qpad, all emitted up
    front into resident exact-sized SBUF slabs (no pools, no issue gating).
  - QK matmuls accumulate all requests into one [128, Wmax] PSUM region via a
    zero-padded stationary q (request v's scores land on rows 4v..4v+3);
    requests sorted by descending context so v=0 initializes every column.
  - mask-free softmax: invalid positions have score exactly 0, exp gives 1,
    and a host-provided per-row count is subtracted from the accumulated sum.
    exp runs in two column halves (the PE transposes overlap the second one).
  - PV per group of 4: one matmul per position-chunk with the shared p^T
    chunk stationary and a contiguous rhs over the group's V (up to 512
    cols), ascending chunks. p stays unnormalized until the output stage:
    os = po * (1/sum) fused into the PSUM->SBUF move, then one out DMA per
    group; the host picks each request's [4,128] block.
"""

import os
import sys

import numpy as np
import ml_dtypes

if "/opt/trn_rl_repo" not in sys.path:
    sys.path.insert(0, "/opt/trn_rl_repo")

import concourse.bacc as bacc
import concourse.bass as bass
import concourse.mybir as mybir
import concourse.tile as tile

BF16 = ml_dtypes.bfloat16

SCALE = 0.08838834764831845  # 1/sqrt(128)
B = 32               # requests
KVH = 8              # kv heads == cores
NH = 4               # q heads per kv head (GQA group)
DH = 128             # head dim
BS = 16              # tokens per cache block
NBLOCKS = 4096       # pool blocks
MBS = 128            # max blocks per sequence
GR = 4               # requests per PV group
NG = B // GR         # PV groups
ALIGN = 64           # K slab column alignment (elements)
NKPIECE = 12         # target K piece-DMA count


def _plan(ctx_sorted):
    """Packing offsets shared by host and device builder.
    ctx_sorted: per-virtual-request context lengths, desc order."""
    Wmax = int(min((int(ctx_sorted[0]) + 127) // 128, MBS) * 128)
    exts = [Wmax] + [int(c) for c in ctx_sorted[1:]]
    # column-block-major K packing: (block j, request v) -> (offset, width).
    # All requests' block-j columns are adjacent so block j's scores are
    # complete while later blocks still stream.
    NBLK = (Wmax + 511) // 512
    kofs, o = {}, 0
    for j in range(NBLK):
        for v in range(B):
            w = min(512, exts[v] - j * 512)
            if w <= 0:
                continue
            kofs[(j, v)] = (o, w)
            o += (w + ALIGN - 1) // ALIGN * ALIGN
    TOTK = o
    Cs = [max((int(c) + 127) // 128, 1) for c in ctx_sorted]
    Cmaxs = [max(Cs[GR * g: GR * g + GR]) for g in range(NG)]
    # V packing: per group, chunk-major, only active requests per chunk
    # (requests in-group are desc by C, so actives are a prefix)
    vofs = [0]          # group slab start
    vcofs = []          # per (g, c) chunk-column offset within vd
    for g in range(NG):
        o2 = vofs[-1]
        percol = []
        for c in range(Cmaxs[g]):
            percol.append(o2)
            o2 += sum(1 for r in range(GR) if Cs[GR * g + r] > c) * DH
        vcofs.append(percol)
        vofs.append(o2)
    TOTV = vofs[-1]
    # K piece boundaries in (j, v) packing order, ~equal bytes; the first
    # piece is just request 0's block-0 columns so QK(0) starts ASAP
    keys = sorted(kofs.keys(), key=lambda k: kofs[k][0])
    target = TOTK / NKPIECE
    first_end = kofs[(0, 0)][0] + kofs[(0, 0)][1]
    pieces, start = [(0, first_end)], first_end
    for kk in keys:
        o2, w2 = kofs[kk]
        if o2 < first_end:
            continue
        if o2 + w2 - first_end >= (TOTK - first_end) / NKPIECE * len(pieces):
            pieces.append((start, o2 + w2))
            start = o2 + w2
    if start < TOTK:
        pieces.append((start, TOTK))
    return Wmax, exts, kofs, TOTK, Cs, Cmaxs, vofs, vcofs, TOTV, pieces


def build_core_program(ctx_sorted):
    nc = bacc.Bacc("TRN2", target_bir_lowering=False)
    f32 = mybir.dt.float32
    bf16 = mybir.dt.bfloat16

    Wmax, exts, kofs, TOTK, Cs, Cmaxs, vofs, vcofs, TOTV, pieces = _plan(ctx_sorted)

    ktd = nc.dram_tensor("ktd", [DH, TOTK], bf16, kind="ExternalInput")
    vd = nc.dram_tensor("vd", [DH, TOTV], bf16, kind="ExternalInput")
    qcid = nc.dram_tensor("qcid", [DH, B * NH + 128], bf16, kind="ExternalInput")
    corrd = nc.dram_tensor("corr", [128, 1], f32, kind="ExternalInput")
    out = nc.dram_tensor("out", [128, GR * DH], f32, kind="ExternalOutput")

    Exp = mybir.ActivationFunctionType.Exp
    NT = Wmax // 128           # 128-position chunks
    NTH = NT // 2 if NT >= 2 else NT

    with tile.TileContext(nc) as tc:
        with (
            tc.tile_pool(name="const", bufs=1) as cpool,
            tc.tile_pool(name="outs", bufs=4) as ospool,
        ):
            qpad_sb = cpool.tile([DH, B * 128], bf16)
            qcid_sb = cpool.tile([DH, B * NH + 128], bf16)  # compact q | ident
            corr_sb = cpool.tile([128, 1], f32)
            kt_all = cpool.tile([128, TOTK], bf16)
            vts = [
                cpool.tile([128, vofs[g + 1] - vofs[g]], bf16, name=f"vt{g}")
                for g in range(NG)
            ]
            p_sb = cpool.tile([128, Wmax], bf16)
            pt_sb = cpool.tile([128, NT, 128], bf16)
            NBLK = (Wmax + 511) // 512
            sums_b = [cpool.tile([128, 1], f32, name=f"sums{j}") for j in range(NBLK)]
            sums = cpool.tile([128, 1], f32)
            recip = cpool.tile([128, 1], f32)

            # ---- all input DMAs up front (single issue queue: a second HWDGE
            # queue makes the engines round-robin V against K and slows QK).
            # qpad (1MB, mostly zeros) is built on-chip: 32KB compact q +
            # gpsimd memset + 32 tiny expansion copies on idle engines.
            # first K piece (request 0, block 0) ahead of everything, then
            # the small constants -- ident/corr are needed mid-stream by the
            # first transposes and sums correction and must not queue behind
            # the whole K stream
            nc.sync.dma_start(
                kt_all[:, pieces[0][0]:pieces[0][1]],
                ktd[:, pieces[0][0]:pieces[0][1]],
            )
            nc.sync.dma_start(qcid_sb[:], qcid[:])
            nc.sync.dma_start(corr_sb[:], corrd[:])
            # memset in 4 range-chunks so QK(0) waits only the first chunk
            QW = B * 128 // 4
            for j in range(4):
                nc.gpsimd.memset(qpad_sb[:, j * QW:(j + 1) * QW], 0.0)
            for v in range(B):
                src = qcid_sb[:, NH * v: NH * (v + 1)]
                dst = qpad_sb[:, v * 128 + NH * v: v * 128 + NH * v + NH]
                if v % 2 == 0:
                    nc.vector.tensor_copy(dst, src)
                else:
                    nc.scalar.copy(dst, src)
            for (a, bnd) in pieces[1:]:
                nc.sync.dma_start(kt_all[:, a:bnd], ktd[:, a:bnd])
            for g in range(NG):
                nc.sync.dma_start(vts[g][:], vd[:, vofs[g]:vofs[g + 1]])

            # ---- QK into one PSUM region; then per-512-block exp + transposes
            # interleaved with PV for the first NEARLY groups (their V has
            # arrived), so the PE never idles between QK and PV. All PSUM
            # pools coexist: scores 8KB + tp 0.5KB + 2x po 2KB = 12.5KB.
            NEARLY = 2

            def pv_mm(popool_tiles, g, c, first, last):
                gC = [Cs[GR * g + r] for r in range(GR)]
                active = sum(1 for x in gC if x > c)
                o3 = vcofs[g][c] - vofs[g]
                nc.tensor.matmul(
                    popool_tiles[g][:, 0:active * DH],
                    lhsT=pt_sb[:, c, :],
                    rhs=vts[g][:, o3: o3 + active * DH],
                    start=first,
                    stop=last,
                )

            def emit_os(po, g):
                os_t = ospool.tile([128, GR * DH], mybir.dt.float32, tag="os")
                nc.vector.tensor_scalar_mul(os_t[:], po[:], recip[:, 0:1])
                # valid rows of group g are the contiguous band 16g..16g+15
                nc.sync.dma_start(
                    out[16 * g: 16 * (g + 1), :], os_t[16 * g: 16 * (g + 1), :]
                )

            with (
                tc.tile_pool(name="pscore", bufs=1, space="PSUM") as pspool,
                tc.tile_pool(name="ptr", bufs=2, space="PSUM") as tppool,
                tc.tile_pool(name="pout", bufs=2, space="PSUM") as popool,
            ):
                # one PSUM tile PER column block: block j's exp then has no
                # (coarse, tile-level) dependency on block j+1's QK writes,
                # so softmax/transposes/early-PV overlap the remaining K
                # stream instead of waiting for all of QK
                sc_t = [
                    pspool.tile([128, 512], f32, name=f"sc{j}")
                    for j in range(NBLK)
                ]
                early_po = {
                    g: popool.tile([128, GR * DH], mybir.dt.float32,
                                   name=f"po{g}", tag="po")
                    for g in range(NEARLY)
                }

                def early_pv(cc):
                    for g in range(NEARLY):
                        if cc < Cmaxs[g]:
                            pv_mm(early_po, g, cc, cc == 0,
                                  cc == Cmaxs[g] - 1)

                def transp_block(blk):
                    c0 = blk * 512
                    bw = min(512, Wmax - c0)
                    for cc in range(c0 // 128, (c0 + bw) // 128):
                        tp = tppool.tile([128, 128], bf16, tag="tp")
                        nc.tensor.transpose(
                            tp[:], p_sb[:, cc * 128:(cc + 1) * 128],
                            qcid_sb[:, B * NH:B * NH + 128],
                        )
                        if cc % 2 == 0:
                            nc.vector.tensor_copy(pt_sb[:, cc, :], tp[:])
                        else:
                            nc.scalar.copy(pt_sb[:, cc, :], tp[:])
                        # PV lags the transposes by one chunk so the PE does
                        # transp(cc) while the copy of pt(cc-1) completes
                        if cc >= 1:
                            early_pv(cc - 1)

                for blk in range(NBLK):
                    c0 = blk * 512
                    bw = min(512, Wmax - c0)
                    active = [v for v in range(B) if (blk, v) in kofs]
                    for v in active:
                        o2, w2 = kofs[(blk, v)]
                        nc.tensor.matmul(
                            sc_t[blk][:, 0:w2],
                            lhsT=qpad_sb[:, v * 128:(v + 1) * 128],
                            rhs=kt_all[:, o2: o2 + w2],
                            start=(v == 0),
                            stop=(v == active[-1]),
                        )
                    nc.scalar.activation(
                        p_sb[:, c0:c0 + bw], sc_t[blk][:, 0:bw], Exp,
                        accum_out=sums_b[blk][:, 0:1],
                    )
                    # transposes for block-1 run while exp(blk) retires; the
                    # PE meanwhile streamed QK(blk) during exp(blk-1)
                    if blk >= 1:
                        transp_block(blk - 1)
                    if blk == 0:
                        nc.vector.tensor_tensor(
                            out=sums[:], in0=sums_b[0][:], in1=corr_sb[:],
                            op=mybir.AluOpType.subtract,
                        )
                    else:
                        nc.vector.tensor_tensor(
                            out=sums[:], in0=sums[:], in1=sums_b[blk][:],
                            op=mybir.AluOpType.add,
                        )
                transp_block(NBLK - 1)
                early_pv((Wmax // 128) - 1)
                nc.vector.reciprocal(recip[:], sums[:])
                for g in range(NEARLY):
                    emit_os(early_po[g], g)

                # ---- remaining groups, V-arrival paced
                for g in range(NEARLY, NG):
                    Cmax = Cmaxs[g]
                    po = popool.tile([128, GR * DH], mybir.dt.float32, tag="po")
                    pot = {g: po}
                    for c in range(Cmax):
                        pv_mm(pot, g, c, c == 0, c == Cmax - 1)
                    emit_os(po, g)

    nc.compile()
    return nc


def _host_inputs(q, k, v, k_cache, v_cache, slot_mapping, block_tables, context_lens):
    """Scatter update, per-request gather/trim (zeroing beyond ctx), packed
    per-core slab layout."""
    D = KVH * DH
    kc = np.asarray(k_cache, dtype=np.float32).reshape(NBLOCKS * BS, D).copy()
    vc = np.asarray(v_cache, dtype=np.float32).reshape(NBLOCKS * BS, D).copy()
    slot = np.asarray(slot_mapping, dtype=np.int64)
    keep = slot >= 0
    kc[slot[keep]] = np.asarray(k, dtype=np.float32).reshape(B, D)[keep]
    vc[slot[keep]] = np.asarray(v, dtype=np.float32).reshape(B, D)[keep]
    kc = kc.reshape(NBLOCKS, BS, KVH, DH)
    vc = vc.reshape(NBLOCKS, BS, KVH, DH)

    bt = np.asarray(block_tables, dtype=np.int64)
    ctx = np.asarray(context_lens, dtype=np.int64)
    qf = np.asarray(q, dtype=np.float32)

    perm = np.argsort(-ctx, kind="stable")  # virtual v -> physical b
    ctx_sorted = ctx[perm].astype(int)
    Wmax, exts, kofs, TOTK, Cs, Cmaxs, vofs, vcofs, TOTV, pieces = _plan(ctx_sorted)
    NBLK = (Wmax + 511) // 512

    KT = np.zeros((KVH, DH, TOTK), dtype=np.float32)
    VD = np.zeros((KVH, 128, TOTV), dtype=np.float32)
    for vv in range(B):
        b = perm[vv]
        cl = int(ctx_sorted[vv])
        nb = int(min((cl + BS - 1) // BS, MBS))
        P = nb * BS
        kseg = kc[bt[b, :nb]]  # [nb, 16, 8, 128]
        vseg = vc[bt[b, :nb]]
        kt_v = np.transpose(kseg, (2, 3, 0, 1)).reshape(KVH, DH, P)
        # K^T in column-block-major packing; only pos < ctx (rest stays 0)
        for j in range(NBLK):
            if (j, vv) not in kofs:
                continue
            o2, w2 = kofs[(j, vv)]
            lo = j * 512
            hi = min(lo + w2, cl)
            if hi > lo:
                KT[:, :, o2: o2 + hi - lo] = kt_v[:, :, lo:hi]
        # V: keep pos < ctx, pad to C*128 chunks, chunk-major within group
        C = Cs[vv]
        vpad = np.zeros((C * 128, KVH, DH), dtype=np.float32)
        vpad[:cl] = vseg.reshape(P, KVH, DH)[:cl]
        g, r = vv // GR, vv % GR
        vch = np.transpose(vpad.reshape(C, 128, KVH, DH), (2, 1, 0, 3))  # [8,128p,C,128d]
        for c in range(C):
            vo = vcofs[g][c] + r * DH  # r < active(c) since C desc in group
            VD[:, :, vo:vo + DH] = vch[:, :, c, :]

    KT = KT.astype(BF16)
    VD = VD.astype(BF16)

    # softmax sum correction: row 4v+h gets (Wmax - ctx) spurious exp(0)=1
    corr = np.repeat((Wmax - ctx_sorted).astype(np.float32), NH).reshape(128, 1)
    ident = np.eye(128, dtype=np.float32).astype(BF16)

    in_maps = []
    for kh in range(KVH):
        qcid = np.empty((DH, B * NH + 128), dtype=np.float32)
        for vv in range(B):
            qcid[:, NH * vv: NH * (vv + 1)] = (
                qf[perm[vv], NH * kh: NH * (kh + 1), :].T * SCALE
            )
        qcid[:, B * NH:] = np.eye(128, dtype=np.float32)
        in_maps.append({
            "ktd": np.ascontiguousarray(KT[kh]),
            "vd": np.ascontiguousarray(VD[kh]),
            "qcid": qcid.astype(BF16),
            "corr": corr,
        })
    return in_maps, perm, ctx_sorted


def kernel(q, k, v, k_cache, v_cache, slot_mapping, block_tables, context_lens):
    from concourse.bass_utils import run_bass_kernel_spmd

    in_maps, perm, ctx_sorted = _host_inputs(
        q, k, v, k_cache, v_cache, slot_mapping, block_tables, context_lens
    )
    nc = build_core_program(list(ctx_sorted))
    core_ids = list(range(KVH))
    res = run_bass_kernel_spmd(
        nc, in_maps, core_ids,
        trace=bool(int(os.environ.get("KERNEL_TRACE", "0"))),
        tmpdir=os.environ.get("KERNEL_TMPDIR") or None,
    )
    kernel.last_results = res
    outs = res.results
    full = np.empty((B, KVH * NH, DH), dtype=np.float32)
    for kh in range(KVH):
        o = np.asarray(outs[kh]["out"], dtype=np.float32)  # [128, GR*DH]
        for vv in range(B):
            r = vv % GR
            blk = o[NH * vv: NH * vv + NH, r * DH:(r + 1) * DH]
            full[perm[vv], NH * kh: NH * (kh + 1), :] = blk
    return full
